# revision 1
# baseline (speedup 1.0000x reference)
"""Trainium2 bit-packing kernel (ConsolidateBits).

Input : x (4096, 32768) float32, uniform [0,1).
Output: (4096, 1024) uint32 — every 32 consecutive values along the last
dim packed into one word, bit i = (x > 0.5) at offset i.

Sharding: data-parallel over the batch dim, 512 rows per core, 8 cores.

Per-core pipeline (~64MB DMA-in per core is the roofline, ~190us):
  DMA  : 16 tiles of [128 part x 8192] f32 (4MB contiguous each)
  DVE  : cmp_lo = (x[seg8 0:4] > 0.5) * 1   -> bf16   (TSP, 2x_2p)
         cmp_hi = (x[seg8 4:8] > 0.5) * 16  -> bf16   (TSP, 2x_2p)
         M1 o1   = lo + hi                   in {0,1,16,17}  (TT bf16, 2x_1p)
         M2 o2   = o1[2:4]*4  + o1[0:2]      (STT, 1x)
         M3 byte = o2[1]*2    + o2[0]        (STT, 1x)
         M4 half = byte_odd*256 + byte_even  -> int32 (STT, 1x)
         M5 word = (half_odd << 16) | half_even      (STT int, 1x)
  DMA  : 16 tiles of [128 x 256] i32 out (viewed uint32 on host)
"""

import sys

if "/opt/trn_rl_repo" not in sys.path:
    sys.path.insert(0, "/opt/trn_rl_repo")

import numpy as np

import concourse.bass as bass  # noqa: F401
import concourse.bacc as bacc
import concourse.mybir as mybir
from concourse.tile import TileContext
from concourse.alu_op_type import AluOpType as A
from concourse.bass_utils import run_bass_kernel_spmd

P = 128
N_CORES = 8
ROWS, COLS = 4096, 32768
ROWS_PER_CORE = ROWS // N_CORES   # 512
F = 8192                          # free-dim elements per partition per tile
NTILES = (ROWS_PER_CORE * COLS) // (P * F)  # 16


def build(ntiles: int = NTILES, free: int = F, gp_cmp_tiles: int = NTILES,
          reps: int = 1, tail_split: int = 4):
    """gp_cmp_tiles: how many tiles run their compares on GPSIMD instead
    of DVE (load balancing; 0 = all on DVE).
    reps: process the whole input `reps` times (benchmarking only —
    lets wall-clock differencing resolve the per-pass kernel time).
    tail_split: split the LAST tile into this many column sub-tiles so the
    serial compute chain after the final DMA is ~tail_split x shorter."""
    nc = bacc.Bacc("TRN2", target_bir_lowering=False)
    x = nc.dram_tensor(
        "x", [ntiles * P, free], mybir.dt.float32, kind="ExternalInput"
    )
    # int32 throughout the bitvec path (walrus: bitvec ops cannot cast);
    # reinterpreted as uint32 on the host.
    y = nc.dram_tensor(
        "y", [ntiles * P, free // 32], mybir.dt.int32, kind="ExternalOutput"
    )
    xr = x[:, :].rearrange("(t p) f -> t p f", p=P)
    yr = y[:, :].rearrange("(t p) w -> t p w", p=P)

    f32, bf16, i32 = mybir.dt.float32, mybir.dt.bfloat16, mybir.dt.int32

    with TileContext(nc) as tc:
        with (
            tc.tile_pool(name="consts", bufs=1) as cpool,
            tc.tile_pool(name="pool", bufs=2) as big_pool,
            tc.tile_pool(name="subpool", bufs=3) as sub_pool,
        ):
            # Walrus requires bitvec-op scalars to be integer-typed and
            # match src/dst dtype; immediates lower as f32, so keep the
            # shift amount in a per-partition int32 const AP.
            shift16 = cpool.tile([P, 1], i32)
            nc.vector.memset(shift16[:], 16)

            ts = max(1, tail_split)
            assert free % (32 * ts) == 0
            work = []
            for t in range(ntiles - 1):
                work.append((t, 0, free))
            if ts >= 4 and free % 16 == 0:
                # descending widths: the last (smallest) piece bounds the
                # serial compute-chain latency after the final DMA lands
                widths = [free * w // 16 for w in (8, 4, 2, 2)]
            else:
                widths = [free // ts] * ts
            col = 0
            for w in widths:
                work.append((ntiles - 1, col, w))
                col += w
            assert col == free
            work = work * reps

            for t, col0, fw in work:
                # sub-tiles (tail split) get their own, deeper pool so the
                # final small DMAs aren't gated on big-tile slot release;
                # half-width pieces still fit the big pool's slots
                pool = big_pool if fw >= free // 2 else sub_pool
                xt = pool.tile([P, fw], f32, tag="xt")
                nc.sync.dma_start(xt[:], xr[t][:, col0 : col0 + fw])
                xv = xt[:].rearrange("p (s m) -> p s m", m=8)

                # Late tiles' compares go to GPSIMD: at stream end the DVE
                # is the tail's critical path, so keep its residual work low.
                cmp_eng = nc.gpsimd if t >= ntiles - gp_cmp_tiles else nc.vector

                # lo = (x>0.5)*1 on seg8[0:4], hi = (x>0.5)*16 on seg8[4:8]
                lo = pool.tile([P, fw // 2], bf16, tag="lo")
                hi = pool.tile([P, fw // 2], bf16, tag="hi")
                lov = lo[:].rearrange("p (s m) -> p s m", m=4)
                hiv = hi[:].rearrange("p (s m) -> p s m", m=4)
                cmp_eng.tensor_scalar(
                    out=lov, in0=xv[:, :, 0:4], scalar1=0.5, scalar2=None,
                    op0=A.is_gt,
                )
                cmp_eng.tensor_scalar(
                    out=hiv, in0=xv[:, :, 4:8], scalar1=0.5, scalar2=16.0,
                    op0=A.is_gt, op1=A.mult,
                )

                # M1: o1[s,m] = b[8s+m] + 16*b[8s+m+4]   {0,1,16,17} bf16
                o1 = pool.tile([P, fw // 2], bf16, tag="o1")
                nc.vector.tensor_tensor(
                    out=o1[:], in0=lo[:], in1=hi[:], op=A.add
                )

                # M2: o2[s,m] = o1[s,m] + 4*o1[s,m+2]    m in [0,2)
                o2 = pool.tile([P, fw // 4], f32, tag="o2")
                o1s = o1[:].rearrange("p (s m) -> p s m", m=4)
                o2v = o2[:].rearrange("p (s m) -> p s m", m=2)
                nc.vector.scalar_tensor_tensor(
                    out=o2v, in0=o1s[:, :, 2:4], scalar=4.0, in1=o1s[:, :, 0:2],
                    op0=A.mult, op1=A.add,
                )

                # M3: byte[s] = o2[s,0] + 2*o2[s,1]      0..255
                byt = pool.tile([P, fw // 8], f32, tag="byt")
                o2s = o2[:].rearrange("p (s m) -> p s m", m=2)
                nc.vector.scalar_tensor_tensor(
                    out=byt[:].rearrange("p (s one) -> p s one", one=1),
                    in0=o2s[:, :, 1:2], scalar=2.0, in1=o2s[:, :, 0:1],
                    op0=A.mult, op1=A.add,
                )

                # M4: half[k] = byte[2k] + 256*byte[2k+1] -> int32 (<=65535)
                half = pool.tile([P, fw // 16], i32, tag="half")
                bys = byt[:].rearrange("p (k h) -> p k h", h=2)
                nc.vector.scalar_tensor_tensor(
                    out=half[:].rearrange("p (k one) -> p k one", one=1),
                    in0=bys[:, :, 1:2], scalar=256.0, in1=bys[:, :, 0:1],
                    op0=A.mult, op1=A.add,
                )

                # M5: word[w] = (half[2w+1] << 16) | half[2w]
                wt = pool.tile([P, fw // 32], i32, tag="wt")
                hs = half[:].rearrange("p (w h) -> p w h", h=2)
                nc.vector.scalar_tensor_tensor(
                    out=wt[:].rearrange("p (w one) -> p w one", one=1),
                    in0=hs[:, :, 1:2], scalar=shift16[:], in1=hs[:, :, 0:1],
                    op0=A.logical_shift_left, op1=A.bitwise_or,
                )

                nc.sync.dma_start(yr[t][:, col0 // 32 : (col0 + fw) // 32], wt[:])

    nc.compile()
    return nc


_NC_CACHE = {}


def _get_nc():
    if "nc" not in _NC_CACHE:
        _NC_CACHE["nc"] = build()
    return _NC_CACHE["nc"]


def _shard(x: np.ndarray):
    return [
        np.ascontiguousarray(
            x[i * ROWS_PER_CORE : (i + 1) * ROWS_PER_CORE].reshape(NTILES * P, F)
        )
        for i in range(N_CORES)
    ]


def run(x: np.ndarray, trace: bool = False):
    """Run the SPMD kernel; returns (full_output, BassKernelResults)."""
    nc = _get_nc()
    in_maps = [{"x": s} for s in _shard(x)]
    res = run_bass_kernel_spmd(nc, in_maps, core_ids=list(range(N_CORES)), trace=trace)
    parts = [
        np.asarray(m["y"]).view(np.uint32).reshape(ROWS_PER_CORE, COLS // 32)
        for m in res.results
    ]
    return np.concatenate(parts, axis=0), res


def kernel(x: np.ndarray) -> np.ndarray:
    out, _ = run(np.asarray(x, dtype=np.float32), trace=False)
    return out



# revision 3
# speedup vs baseline: 1.0160x; 1.0160x over previous
"""Trainium2 bit-packing kernel (ConsolidateBits).

Input : x (4096, 32768) float32, uniform [0,1).
Output: (4096, 1024) uint32 — every 32 consecutive values along the last
dim packed into one word, bit i = (x > 0.5) at offset i.

Sharding: data-parallel over the batch dim, 512 rows per core, 8 cores.

Per-core plan (512 x 32768 f32 = 16 row-tiles of [128 part x 8192]):
  DMA-in : every tile's columns are split across the three DMA queues
           (SP / Act / Pool-SWDGE), each modeled at ~332 GB/s, so arrivals
           advance in lockstep with compute consumption.
  DVE    : 8 scaled compares per compute segment,
           c_j = (x[8s+j] > 0.5) * 2^j -> uint8  (is_gt+mult, 2x_2p).
           c_j[s] is byte s's bit-j contribution. Tiles 1..14 are processed
           as 7 two-tile macro segments to amortize instruction overhead.
  Pool   : radix-2 add tree on int32 views of the uint8 lanes — each int32
           add carries FOUR byte-lanes at once (all partial sums <= 255 so
           no carry ever crosses a lane). The final level's int32 result IS
           the packed little-endian word array.
  DMA-out: one contiguous int32 store per segment, issued with a lag so it
           never blocks a queue head.
"""

import sys

if "/opt/trn_rl_repo" not in sys.path:
    sys.path.insert(0, "/opt/trn_rl_repo")

import numpy as np

import concourse.bass as bass  # noqa: F401
import concourse.bacc as bacc
import concourse.mybir as mybir
from concourse.tile import TileContext
from concourse.alu_op_type import AluOpType as A
from concourse.bass_utils import run_bass_kernel_spmd

P = 128
N_CORES = 8
ROWS, COLS = 4096, 32768
ROWS_PER_CORE = ROWS // N_CORES   # 512
F = 8192                          # free-dim elements per partition per tile
NTILES = (ROWS_PER_CORE * COLS) // (P * F)  # 16

f32 = mybir.dt.float32
i32 = mybir.dt.int32
u8 = mybir.dt.uint8


def default_plan(pool_map, t0_widths, t15_widths, macro):
    """Returns (segments, chunks).

    segments: consumption-ordered (tiles, col0, width, new_xt) where
    `tiles` is the tuple of row-tiles the segment's xt allocation spans
    (compare/tree instructions cover the whole allocation for macro
    segments). col0/width are within the allocation.
    chunks: per segment, list of (alloc_col0, width, queue) DMA pieces.
    """
    segs = []
    chunks = []

    def hw_split(alloc0, tile, tcol, w, pw_q=None):
        """Chunks (alloc_c0, tile, tile_c0, width, queue) covering
        [tcol, tcol+w) of `tile`; optional Pool lead chunk of pw_q cols."""
        ch = []
        if pw_q:
            ch.append((alloc0, tile, tcol, pw_q, "gpsimd"))
            alloc0 += pw_q
            tcol += pw_q
            w -= pw_q
        ch.append((alloc0, tile, tcol, w // 2, "sync"))
        ch.append((alloc0 + w // 2, tile, tcol + w // 2, w - w // 2, "scalar"))
        return ch

    # tile 0: split segments sharing one full-tile allocation
    assert sum(t0_widths) == F
    col = 0
    for w in t0_widths:
        segs.append(((0,), w, col))
        chunks.append(hw_split(col, 0, col, w))
        col += w

    # tiles 1..14: two-tile macro segments (or singles if macro=1)
    t = 1
    while t < NTILES - 1:
        span = min(macro, NTILES - 1 - t)
        tiles = tuple(range(t, t + span))
        segs.append((tiles, span * F, 0))
        ch = []
        for k, tt in enumerate(tiles):
            pw = pool_map.get(tt, 0)
            ch += hw_split(k * F, tt, 0, F, pw or None)
        chunks.append(ch)
        t += span

    # tile 15: split segments sharing one full-tile allocation
    assert sum(t15_widths) == F
    col = 0
    for w in t15_widths:
        segs.append(((15,), w, col))
        chunks.append(hw_split(col, 15, col, w))
        col += w
    return segs, chunks


def build(
    dve_trees=(),            # segment indices with tree on DVE
    out_lag=3,               # out-DMA issued this many segments late
    lead=4,                  # in-DMA issued this many segments early
    pool_tiles=(3, 4, 5, 6, 7, 8, 9, 10, 11, 12, 13, 14),
    pool_width=2176,
    pool_head={1: 4096, 2: 4096},
    tail_outs_on_pool=0,
    t0_widths=(2048, 2048, 2048, 2048),
    t15_widths=(4096, 2048, 1024, 1024),
    macro=1,
    out_batch=2,
    xt_bufs=4,
    c_bufs=3,
    mid_bufs=2,
    b_bufs=6,
    pool_map=None,
):
    if pool_map is None:
        pool_map = dict(pool_head)
        pool_map.update({t: pool_width for t in pool_tiles})
    else:
        pool_map = {int(k): v for k, v in pool_map.items()}
    segs, chunks = default_plan(pool_map, tuple(t0_widths),
                                tuple(t15_widths), macro)
    n_seg = len(segs)

    nc = bacc.Bacc("TRN2", target_bir_lowering=False)
    x = nc.dram_tensor("x", [NTILES * P, F], f32, kind="ExternalInput")
    y = nc.dram_tensor("y", [NTILES * P, F // 32], i32, kind="ExternalOutput")
    xr = x[:, :].rearrange("(t p) f -> t p f", p=P)
    yr = y[:, :].rearrange("(t p) w -> t p w", p=P)

    with TileContext(nc) as tc:
        with (
            tc.tile_pool(name="xt", bufs=xt_bufs) as xt_pool,
            tc.tile_pool(name="c", bufs=c_bufs) as c_pool,
            tc.tile_pool(name="mid", bufs=mid_bufs) as mid_pool,
            tc.tile_pool(name="b", bufs=b_bufs) as b_pool,
        ):
            xts = [None] * n_seg
            shared_xt = {}
            outs = [None] * n_seg
            pending_b = [None]
            out_rr = 0

            def issue_in(si):
                tiles, fw, tcol = segs[si]
                if tiles[0] not in (0, NTILES - 1):
                    xt = xt_pool.tile([P, len(tiles) * F], f32, tag="xt",
                                      name=f"xt_s{si}")
                else:
                    # head/tail tiles: one shared full-tile allocation;
                    # segments DMA into their column ranges
                    t = tiles[0]
                    if t not in shared_xt:
                        shared_xt[t] = xt_pool.tile(
                            [P, F], f32, tag="xt", name=f"xth{t}")
                    xt = shared_xt[t]
                for c0, tt, tc0, w, q in chunks[si]:
                    getattr(nc, q).dma_start(
                        xt[:, c0:c0 + w], xr[tt][:, tc0:tc0 + w])
                xts[si] = xt

            def issue_out(si):
                if outs[si] is None:
                    return
                b, tiles, tcol, nw, q = outs[si]
                if len(tiles) == 1:
                    dst = yr[tiles[0]][:, tcol // 32: tcol // 32 + nw]
                    src = b[:]
                else:
                    h = len(tiles)
                    dst = y[P * tiles[0]: P * (tiles[0] + h), :].rearrange(
                        "(h p) w -> p h w", p=P)
                    src = b[:].rearrange("p (h w) -> p h w", h=h)
                getattr(nc, q).dma_start(dst, src)

            for si in range(min(lead, n_seg)):
                issue_in(si)

            for si in range(n_seg):
                tiles, fw, tcol = segs[si]
                nb = fw // 8
                nw = fw // 32
                xt = xts[si]
                if tiles[0] in (0, NTILES - 1):
                    xv = xt[:, tcol:tcol + fw].rearrange(
                        "p (s m) -> p s m", m=8)
                else:
                    xv = xt[:].rearrange("p (s m) -> p s m", m=8)

                # 8 scaled compares: c_j = (x[8s+j] > 0.5) * 2^j  -> uint8
                c = c_pool.tile([P, 8 * nb], u8, tag="c", name=f"c{si}")
                cv = c[:].rearrange("p (j s) -> p j s", j=8)
                for j in range(8):
                    nc.vector.tensor_scalar(
                        out=cv[:, j], in0=xv[:, :, j],
                        scalar1=0.5, scalar2=float(1 << j),
                        op0=A.is_gt, op1=A.mult,
                    )

                # add tree on int32 views (4 uint8 byte-lanes per element);
                # the L3 int32 result IS the packed little-endian word array
                eng = nc.vector if si in dve_trees else nc.gpsimd
                ci = c[:].bitcast(i32).rearrange("p (j s) -> p j s", j=8)
                t1 = mid_pool.tile([P, nb], i32, tag="t1", name=f"t1_{si}")
                t1v = t1[:].rearrange("p (j s) -> p j s", j=4)
                eng.tensor_tensor(
                    out=t1v, in0=ci[:, 0:8:2], in1=ci[:, 1:8:2], op=A.add)
                t2 = mid_pool.tile([P, nb // 2], i32, tag="t2", name=f"t2_{si}")
                t2v = t2[:].rearrange("p (j s) -> p j s", j=2)
                t1b = t1[:].rearrange("p (j s) -> p j s", j=4)
                eng.tensor_tensor(
                    out=t2v, in0=t1b[:, 0:4:2], in1=t1b[:, 1:4:2], op=A.add)
                # mid tiles: pair consecutive tiles into one shared b tile
                # so a single out-DMA stores both (y rows are contiguous)
                mid = tiles[0] not in (0, NTILES - 1)
                pair = (out_batch == 2 and mid and len(tiles) == 1
                        and 1 <= tiles[0] <= 14)
                if pair and tiles[0] % 2 == 1:
                    b = b_pool.tile([P, 2 * nw], i32, tag="b", name=f"b{si}")
                    pending_b[0] = b
                    bslice = b[:, 0:nw]
                elif pair:
                    b = pending_b[0]
                    bslice = b[:, nw:2 * nw]
                else:
                    b = b_pool.tile([P, nw], i32, tag="b", name=f"b{si}")
                    bslice = b[:]
                t2b = t2[:].rearrange("p (j s) -> p j s", j=2)
                eng.tensor_tensor(
                    out=bslice.rearrange("p (one s) -> p one s", one=1),
                    in0=t2b[:, 0:1], in1=t2b[:, 1:2], op=A.add)

                if si >= n_seg - tail_outs_on_pool:
                    oq = "gpsimd"
                else:
                    oq = "sync" if out_rr % 2 == 0 else "scalar"
                if pair and tiles[0] % 2 == 1:
                    outs[si] = None          # store happens with the pair's 2nd
                else:
                    out_rr += 1
                    if pair:
                        outs[si] = (b, (tiles[0] - 1, tiles[0]), 0, 2 * nw, oq)
                    else:
                        outs[si] = (b, tiles, tcol, nw, oq)

                if si + lead < n_seg:
                    issue_in(si + lead)
                if si >= out_lag:
                    issue_out(si - out_lag)

            for si in range(n_seg - out_lag, n_seg):
                issue_out(si)

    nc.compile()
    return nc


_NC_CACHE = {}


def _get_nc():
    if "nc" not in _NC_CACHE:
        _NC_CACHE["nc"] = build()
    return _NC_CACHE["nc"]


def _shard(x: np.ndarray):
    return [
        np.ascontiguousarray(
            x[i * ROWS_PER_CORE: (i + 1) * ROWS_PER_CORE].reshape(NTILES * P, F)
        )
        for i in range(N_CORES)
    ]


def run(x: np.ndarray, trace: bool = False):
    nc = _get_nc()
    in_maps = [{"x": s} for s in _shard(x)]
    res = run_bass_kernel_spmd(nc, in_maps, core_ids=list(range(N_CORES)), trace=trace)
    parts = [
        np.asarray(m["y"]).view(np.uint32).reshape(ROWS_PER_CORE, COLS // 32)
        for m in res.results
    ]
    return np.concatenate(parts, axis=0), res


def kernel(x: np.ndarray) -> np.ndarray:
    out, _ = run(np.asarray(x, dtype=np.float32), trace=False)
    return out


# revision 4
# speedup vs baseline: 1.0178x; 1.0018x over previous
"""Trainium2 bit-packing kernel (ConsolidateBits).

Input : x (4096, 32768) float32, uniform [0,1).
Output: (4096, 1024) uint32 — every 32 consecutive values along the last
dim packed into one word, bit i = (x > 0.5) at offset i.

Sharding: data-parallel over the batch dim, 512 rows per core, 8 cores.

Per-core plan (512 x 32768 f32 = 16 row-tiles of [128 part x 8192]):
  DMA-in : every tile's columns are split across the three DMA queues
           (SP / Act / Pool-SWDGE), each modeled at ~332 GB/s, so arrivals
           advance in lockstep with compute consumption.
  DVE    : 8 scaled compares per compute segment,
           c_j = (x[8s+j] > 0.5) * 2^j -> uint8  (is_gt+mult, 2x_2p).
           c_j[s] is byte s's bit-j contribution. Tiles 1..14 are processed
           as 7 two-tile macro segments to amortize instruction overhead.
  Pool   : radix-2 add tree on int32 views of the uint8 lanes — each int32
           add carries FOUR byte-lanes at once (all partial sums <= 255 so
           no carry ever crosses a lane). The final level's int32 result IS
           the packed little-endian word array.
  DMA-out: one contiguous int32 store per segment, issued with a lag so it
           never blocks a queue head.
"""

import sys

if "/opt/trn_rl_repo" not in sys.path:
    sys.path.insert(0, "/opt/trn_rl_repo")

import numpy as np

import concourse.bass as bass  # noqa: F401
import concourse.bacc as bacc
import concourse.mybir as mybir
from concourse.tile import TileContext
from concourse.alu_op_type import AluOpType as A
from concourse.bass_utils import run_bass_kernel_spmd

P = 128
N_CORES = 8
ROWS, COLS = 4096, 32768
ROWS_PER_CORE = ROWS // N_CORES   # 512
F = 8192                          # free-dim elements per partition per tile
NTILES = (ROWS_PER_CORE * COLS) // (P * F)  # 16

f32 = mybir.dt.float32
i32 = mybir.dt.int32
u8 = mybir.dt.uint8


def default_plan(pool_map, t0_widths, t15_widths, macro):
    """Returns (segments, chunks).

    segments: consumption-ordered (tiles, col0, width, new_xt) where
    `tiles` is the tuple of row-tiles the segment's xt allocation spans
    (compare/tree instructions cover the whole allocation for macro
    segments). col0/width are within the allocation.
    chunks: per segment, list of (alloc_col0, width, queue) DMA pieces.
    """
    segs = []
    chunks = []

    def hw_split(alloc0, tile, tcol, w, pw_q=None):
        """Chunks (alloc_c0, tile, tile_c0, width, queue) covering
        [tcol, tcol+w) of `tile`; optional Pool lead chunk of pw_q cols."""
        ch = []
        if pw_q:
            ch.append((alloc0, tile, tcol, pw_q, "gpsimd"))
            alloc0 += pw_q
            tcol += pw_q
            w -= pw_q
        ch.append((alloc0, tile, tcol, w // 2, "sync"))
        ch.append((alloc0 + w // 2, tile, tcol + w // 2, w - w // 2, "scalar"))
        return ch

    # tile 0: split segments sharing one full-tile allocation
    assert sum(t0_widths) == F
    col = 0
    for w in t0_widths:
        segs.append(((0,), w, col))
        chunks.append(hw_split(col, 0, col, w))
        col += w

    # tiles 1..14: two-tile macro segments (or singles if macro=1)
    t = 1
    while t < NTILES - 1:
        span = min(macro, NTILES - 1 - t)
        tiles = tuple(range(t, t + span))
        segs.append((tiles, span * F, 0))
        ch = []
        for k, tt in enumerate(tiles):
            pw = pool_map.get(tt, 0)
            ch += hw_split(k * F, tt, 0, F, pw or None)
        chunks.append(ch)
        t += span

    # tile 15: split segments sharing one full-tile allocation
    assert sum(t15_widths) == F
    col = 0
    for w in t15_widths:
        segs.append(((15,), w, col))
        chunks.append(hw_split(col, 15, col, w))
        col += w
    return segs, chunks


def build(
    dve_trees=(),            # segment indices with tree on DVE
    out_lag=3,               # out-DMA issued this many segments late
    lead=4,                  # in-DMA issued this many segments early
    pool_tiles=(3, 4, 5, 6, 7, 8, 9, 10, 11, 12, 13, 14),
    pool_width=2176,
    pool_head={1: 4096, 2: 4096},
    tail_outs_on_pool=0,
    t0_widths=(2048, 2048, 2048, 2048),
    t15_widths=(4096, 2048, 1792, 256),
    macro=1,
    out_batch=2,
    xt_bufs=4,
    c_bufs=3,
    mid_bufs=2,
    b_bufs=6,
    pool_map=None,
):
    if pool_map is None:
        pool_map = dict(pool_head)
        pool_map.update({t: pool_width for t in pool_tiles})
    else:
        pool_map = {int(k): v for k, v in pool_map.items()}
    segs, chunks = default_plan(pool_map, tuple(t0_widths),
                                tuple(t15_widths), macro)
    n_seg = len(segs)

    nc = bacc.Bacc("TRN2", target_bir_lowering=False)
    x = nc.dram_tensor("x", [NTILES * P, F], f32, kind="ExternalInput")
    y = nc.dram_tensor("y", [NTILES * P, F // 32], i32, kind="ExternalOutput")
    xr = x[:, :].rearrange("(t p) f -> t p f", p=P)
    yr = y[:, :].rearrange("(t p) w -> t p w", p=P)

    with TileContext(nc) as tc:
        with (
            tc.tile_pool(name="xt", bufs=xt_bufs) as xt_pool,
            tc.tile_pool(name="c", bufs=c_bufs) as c_pool,
            tc.tile_pool(name="mid", bufs=mid_bufs) as mid_pool,
            tc.tile_pool(name="b", bufs=b_bufs) as b_pool,
        ):
            xts = [None] * n_seg
            shared_xt = {}
            outs = [None] * n_seg
            pending_b = [None]
            out_rr = 0

            def issue_in(si):
                tiles, fw, tcol = segs[si]
                if tiles[0] not in (0, NTILES - 1):
                    xt = xt_pool.tile([P, len(tiles) * F], f32, tag="xt",
                                      name=f"xt_s{si}")
                else:
                    # head/tail tiles: one shared full-tile allocation;
                    # segments DMA into their column ranges
                    t = tiles[0]
                    if t not in shared_xt:
                        shared_xt[t] = xt_pool.tile(
                            [P, F], f32, tag="xt", name=f"xth{t}")
                    xt = shared_xt[t]
                for c0, tt, tc0, w, q in chunks[si]:
                    getattr(nc, q).dma_start(
                        xt[:, c0:c0 + w], xr[tt][:, tc0:tc0 + w])
                xts[si] = xt

            def issue_out(si):
                if outs[si] is None:
                    return
                b, tiles, tcol, nw, q = outs[si]
                if len(tiles) == 1:
                    dst = yr[tiles[0]][:, tcol // 32: tcol // 32 + nw]
                    src = b[:]
                else:
                    h = len(tiles)
                    dst = y[P * tiles[0]: P * (tiles[0] + h), :].rearrange(
                        "(h p) w -> p h w", p=P)
                    src = b[:].rearrange("p (h w) -> p h w", h=h)
                getattr(nc, q).dma_start(dst, src)

            for si in range(min(lead, n_seg)):
                issue_in(si)

            for si in range(n_seg):
                tiles, fw, tcol = segs[si]
                nb = fw // 8
                nw = fw // 32
                xt = xts[si]
                if tiles[0] in (0, NTILES - 1):
                    xv = xt[:, tcol:tcol + fw].rearrange(
                        "p (s m) -> p s m", m=8)
                else:
                    xv = xt[:].rearrange("p (s m) -> p s m", m=8)

                # 8 scaled compares: c_j = (x[8s+j] > 0.5) * 2^j  -> uint8
                c = c_pool.tile([P, 8 * nb], u8, tag="c", name=f"c{si}")
                cv = c[:].rearrange("p (j s) -> p j s", j=8)
                for j in range(8):
                    nc.vector.tensor_scalar(
                        out=cv[:, j], in0=xv[:, :, j],
                        scalar1=0.5, scalar2=float(1 << j),
                        op0=A.is_gt, op1=A.mult,
                    )

                # add tree on int32 views (4 uint8 byte-lanes per element);
                # the L3 int32 result IS the packed little-endian word array
                eng = nc.vector if si in dve_trees else nc.gpsimd
                ci = c[:].bitcast(i32).rearrange("p (j s) -> p j s", j=8)
                t1 = mid_pool.tile([P, nb], i32, tag="t1", name=f"t1_{si}")
                t1v = t1[:].rearrange("p (j s) -> p j s", j=4)
                eng.tensor_tensor(
                    out=t1v, in0=ci[:, 0:8:2], in1=ci[:, 1:8:2], op=A.add)
                t2 = mid_pool.tile([P, nb // 2], i32, tag="t2", name=f"t2_{si}")
                t2v = t2[:].rearrange("p (j s) -> p j s", j=2)
                t1b = t1[:].rearrange("p (j s) -> p j s", j=4)
                eng.tensor_tensor(
                    out=t2v, in0=t1b[:, 0:4:2], in1=t1b[:, 1:4:2], op=A.add)
                # mid tiles: pair consecutive tiles into one shared b tile
                # so a single out-DMA stores both (y rows are contiguous)
                mid = tiles[0] not in (0, NTILES - 1)
                pair = (out_batch == 2 and mid and len(tiles) == 1
                        and 1 <= tiles[0] <= 14)
                if pair and tiles[0] % 2 == 1:
                    b = b_pool.tile([P, 2 * nw], i32, tag="b", name=f"b{si}")
                    pending_b[0] = b
                    bslice = b[:, 0:nw]
                elif pair:
                    b = pending_b[0]
                    bslice = b[:, nw:2 * nw]
                else:
                    b = b_pool.tile([P, nw], i32, tag="b", name=f"b{si}")
                    bslice = b[:]
                t2b = t2[:].rearrange("p (j s) -> p j s", j=2)
                eng.tensor_tensor(
                    out=bslice.rearrange("p (one s) -> p one s", one=1),
                    in0=t2b[:, 0:1], in1=t2b[:, 1:2], op=A.add)

                if si >= n_seg - tail_outs_on_pool:
                    oq = "gpsimd"
                else:
                    oq = "sync" if out_rr % 2 == 0 else "scalar"
                if pair and tiles[0] % 2 == 1:
                    outs[si] = None          # store happens with the pair's 2nd
                else:
                    out_rr += 1
                    if pair:
                        outs[si] = (b, (tiles[0] - 1, tiles[0]), 0, 2 * nw, oq)
                    else:
                        outs[si] = (b, tiles, tcol, nw, oq)

                if si + lead < n_seg:
                    issue_in(si + lead)
                if si >= out_lag:
                    issue_out(si - out_lag)

            for si in range(n_seg - out_lag, n_seg):
                issue_out(si)

    nc.compile()
    return nc


_NC_CACHE = {}


def _get_nc():
    if "nc" not in _NC_CACHE:
        _NC_CACHE["nc"] = build()
    return _NC_CACHE["nc"]


def _shard(x: np.ndarray):
    return [
        np.ascontiguousarray(
            x[i * ROWS_PER_CORE: (i + 1) * ROWS_PER_CORE].reshape(NTILES * P, F)
        )
        for i in range(N_CORES)
    ]


def run(x: np.ndarray, trace: bool = False):
    nc = _get_nc()
    in_maps = [{"x": s} for s in _shard(x)]
    res = run_bass_kernel_spmd(nc, in_maps, core_ids=list(range(N_CORES)), trace=trace)
    parts = [
        np.asarray(m["y"]).view(np.uint32).reshape(ROWS_PER_CORE, COLS // 32)
        for m in res.results
    ]
    return np.concatenate(parts, axis=0), res


def kernel(x: np.ndarray) -> np.ndarray:
    out, _ = run(np.asarray(x, dtype=np.float32), trace=False)
    return out


# revision 5
# speedup vs baseline: 1.0251x; 1.0072x over previous
"""Trainium2 bit-packing kernel (ConsolidateBits).

Input : x (4096, 32768) float32, uniform [0,1).
Output: (4096, 1024) uint32 — every 32 consecutive values along the last
dim packed into one word, bit i = (x > 0.5) at offset i.

Sharding: data-parallel over the batch dim, 512 rows per core, 8 cores.

Per-core plan (512 x 32768 f32 = 16 row-tiles of [128 part x 8192]):
  DMA-in : every tile's columns are split across the three DMA queues
           (SP / Act / Pool-SWDGE), each modeled at ~332 GB/s, so arrivals
           advance in lockstep with compute consumption.
  DVE    : 8 scaled compares per compute segment,
           c_j = (x[8s+j] > 0.5) * 2^j -> uint8  (is_gt+mult, 2x_2p).
           c_j[s] is byte s's bit-j contribution. Tiles 1..14 are processed
           as 7 two-tile macro segments to amortize instruction overhead.
  Pool   : radix-2 add tree on int32 views of the uint8 lanes — each int32
           add carries FOUR byte-lanes at once (all partial sums <= 255 so
           no carry ever crosses a lane). The final level's int32 result IS
           the packed little-endian word array.
  DMA-out: one contiguous int32 store per segment, issued with a lag so it
           never blocks a queue head.
"""

import sys

if "/opt/trn_rl_repo" not in sys.path:
    sys.path.insert(0, "/opt/trn_rl_repo")

import numpy as np

import concourse.bass as bass  # noqa: F401
import concourse.bacc as bacc
import concourse.mybir as mybir
from concourse.tile import TileContext
from concourse.alu_op_type import AluOpType as A
from concourse.bass_utils import run_bass_kernel_spmd

P = 128
N_CORES = 8
ROWS, COLS = 4096, 32768
ROWS_PER_CORE = ROWS // N_CORES   # 512
F = 8192                          # free-dim elements per partition per tile
NTILES = (ROWS_PER_CORE * COLS) // (P * F)  # 16

f32 = mybir.dt.float32
i32 = mybir.dt.int32
u8 = mybir.dt.uint8


def default_plan(pool_map, t0_widths, t15_widths, macro):
    """Returns (segments, chunks).

    segments: consumption-ordered (tiles, col0, width, new_xt) where
    `tiles` is the tuple of row-tiles the segment's xt allocation spans
    (compare/tree instructions cover the whole allocation for macro
    segments). col0/width are within the allocation.
    chunks: per segment, list of (alloc_col0, width, queue) DMA pieces.
    """
    segs = []
    chunks = []

    def hw_split(alloc0, tile, tcol, w, pw_q=None):
        """Chunks (alloc_c0, tile, tile_c0, width, queue) covering
        [tcol, tcol+w) of `tile`; optional Pool lead chunk of pw_q cols."""
        ch = []
        if pw_q:
            ch.append((alloc0, tile, tcol, pw_q, "gpsimd"))
            alloc0 += pw_q
            tcol += pw_q
            w -= pw_q
        ch.append((alloc0, tile, tcol, w // 2, "sync"))
        ch.append((alloc0 + w // 2, tile, tcol + w // 2, w - w // 2, "scalar"))
        return ch

    # tile 0: split segments sharing one full-tile allocation
    assert sum(t0_widths) == F
    col = 0
    for w in t0_widths:
        segs.append(((0,), w, col))
        chunks.append(hw_split(col, 0, col, w))
        col += w

    # tiles 1..14: two-tile macro segments (or singles if macro=1)
    t = 1
    while t < NTILES - 1:
        span = min(macro, NTILES - 1 - t)
        tiles = tuple(range(t, t + span))
        segs.append((tiles, span * F, 0))
        ch = []
        for k, tt in enumerate(tiles):
            pw = pool_map.get(tt, 0)
            ch += hw_split(k * F, tt, 0, F, pw or None)
        chunks.append(ch)
        t += span

    # tile 15: split segments sharing one full-tile allocation
    assert sum(t15_widths) == F
    col = 0
    for w in t15_widths:
        segs.append(((15,), w, col))
        chunks.append(hw_split(col, 15, col, w))
        col += w
    return segs, chunks


def build(
    dve_trees=(),            # segment indices with tree on DVE
    pool_cmps=(20, 21),      # segment indices with compares on Pool
    out_lag=3,               # out-DMA issued this many segments late
    lead=4,                  # in-DMA issued this many segments early
    pool_tiles=(3, 4, 5, 6, 7, 8, 9, 10, 11, 12, 13, 14),
    pool_width=2176,
    pool_head={1: 4096, 2: 4096},
    tail_outs_on_pool=0,
    t0_widths=(2048, 2048, 2048, 2048),
    t15_widths=(4096, 2048, 1792, 256),
    macro=1,
    out_batch=2,
    xt_bufs=4,
    c_bufs=3,
    mid_bufs=2,
    b_bufs=6,
    pool_map=None,
):
    if pool_map is None:
        pool_map = dict(pool_head)
        pool_map.update({t: pool_width for t in pool_tiles})
    else:
        pool_map = {int(k): v for k, v in pool_map.items()}
    segs, chunks = default_plan(pool_map, tuple(t0_widths),
                                tuple(t15_widths), macro)
    n_seg = len(segs)

    nc = bacc.Bacc("TRN2", target_bir_lowering=False)
    x = nc.dram_tensor("x", [NTILES * P, F], f32, kind="ExternalInput")
    y = nc.dram_tensor("y", [NTILES * P, F // 32], i32, kind="ExternalOutput")
    xr = x[:, :].rearrange("(t p) f -> t p f", p=P)
    yr = y[:, :].rearrange("(t p) w -> t p w", p=P)

    with TileContext(nc) as tc:
        with (
            tc.tile_pool(name="xt", bufs=xt_bufs) as xt_pool,
            tc.tile_pool(name="c", bufs=c_bufs) as c_pool,
            tc.tile_pool(name="mid", bufs=mid_bufs) as mid_pool,
            tc.tile_pool(name="b", bufs=b_bufs) as b_pool,
        ):
            xts = [None] * n_seg
            shared_xt = {}
            outs = [None] * n_seg
            pending_b = [None]
            out_rr = 0

            def issue_in(si):
                tiles, fw, tcol = segs[si]
                if tiles[0] not in (0, NTILES - 1):
                    xt = xt_pool.tile([P, len(tiles) * F], f32, tag="xt",
                                      name=f"xt_s{si}")
                else:
                    # head/tail tiles: one shared full-tile allocation;
                    # segments DMA into their column ranges
                    t = tiles[0]
                    if t not in shared_xt:
                        shared_xt[t] = xt_pool.tile(
                            [P, F], f32, tag="xt", name=f"xth{t}")
                    xt = shared_xt[t]
                for c0, tt, tc0, w, q in chunks[si]:
                    getattr(nc, q).dma_start(
                        xt[:, c0:c0 + w], xr[tt][:, tc0:tc0 + w])
                xts[si] = xt

            def issue_out(si):
                if outs[si] is None:
                    return
                b, tiles, tcol, nw, q = outs[si]
                if len(tiles) == 1:
                    dst = yr[tiles[0]][:, tcol // 32: tcol // 32 + nw]
                    src = b[:]
                else:
                    h = len(tiles)
                    dst = y[P * tiles[0]: P * (tiles[0] + h), :].rearrange(
                        "(h p) w -> p h w", p=P)
                    src = b[:].rearrange("p (h w) -> p h w", h=h)
                getattr(nc, q).dma_start(dst, src)

            for si in range(min(lead, n_seg)):
                issue_in(si)

            for si in range(n_seg):
                tiles, fw, tcol = segs[si]
                nb = fw // 8
                nw = fw // 32
                xt = xts[si]
                if tiles[0] in (0, NTILES - 1):
                    xv = xt[:, tcol:tcol + fw].rearrange(
                        "p (s m) -> p s m", m=8)
                else:
                    xv = xt[:].rearrange("p (s m) -> p s m", m=8)

                # 8 scaled compares: c_j = (x[8s+j] > 0.5) * 2^j  -> uint8
                c = c_pool.tile([P, 8 * nb], u8, tag="c", name=f"c{si}")
                cv = c[:].rearrange("p (j s) -> p j s", j=8)
                cmp_eng = nc.gpsimd if si in pool_cmps else nc.vector
                for j in range(8):
                    cmp_eng.tensor_scalar(
                        out=cv[:, j], in0=xv[:, :, j],
                        scalar1=0.5, scalar2=float(1 << j),
                        op0=A.is_gt, op1=A.mult,
                    )

                # add tree on int32 views (4 uint8 byte-lanes per element);
                # the L3 int32 result IS the packed little-endian word array
                eng = nc.vector if si in dve_trees else nc.gpsimd
                ci = c[:].bitcast(i32).rearrange("p (j s) -> p j s", j=8)
                t1 = mid_pool.tile([P, nb], i32, tag="t1", name=f"t1_{si}")
                t1v = t1[:].rearrange("p (j s) -> p j s", j=4)
                eng.tensor_tensor(
                    out=t1v, in0=ci[:, 0:8:2], in1=ci[:, 1:8:2], op=A.add)
                t2 = mid_pool.tile([P, nb // 2], i32, tag="t2", name=f"t2_{si}")
                t2v = t2[:].rearrange("p (j s) -> p j s", j=2)
                t1b = t1[:].rearrange("p (j s) -> p j s", j=4)
                eng.tensor_tensor(
                    out=t2v, in0=t1b[:, 0:4:2], in1=t1b[:, 1:4:2], op=A.add)
                # mid tiles: pair consecutive tiles into one shared b tile
                # so a single out-DMA stores both (y rows are contiguous)
                mid = tiles[0] not in (0, NTILES - 1)
                pair = (out_batch == 2 and mid and len(tiles) == 1
                        and 1 <= tiles[0] <= 14)
                if pair and tiles[0] % 2 == 1:
                    b = b_pool.tile([P, 2 * nw], i32, tag="b", name=f"b{si}")
                    pending_b[0] = b
                    bslice = b[:, 0:nw]
                elif pair:
                    b = pending_b[0]
                    bslice = b[:, nw:2 * nw]
                else:
                    b = b_pool.tile([P, nw], i32, tag="b", name=f"b{si}")
                    bslice = b[:]
                t2b = t2[:].rearrange("p (j s) -> p j s", j=2)
                eng.tensor_tensor(
                    out=bslice.rearrange("p (one s) -> p one s", one=1),
                    in0=t2b[:, 0:1], in1=t2b[:, 1:2], op=A.add)

                if si >= n_seg - tail_outs_on_pool:
                    oq = "gpsimd"
                else:
                    oq = "sync" if out_rr % 2 == 0 else "scalar"
                if pair and tiles[0] % 2 == 1:
                    outs[si] = None          # store happens with the pair's 2nd
                else:
                    out_rr += 1
                    if pair:
                        outs[si] = (b, (tiles[0] - 1, tiles[0]), 0, 2 * nw, oq)
                    else:
                        outs[si] = (b, tiles, tcol, nw, oq)

                if si + lead < n_seg:
                    issue_in(si + lead)
                if si >= out_lag:
                    issue_out(si - out_lag)

            for si in range(n_seg - out_lag, n_seg):
                issue_out(si)

    nc.compile()
    return nc


_NC_CACHE = {}


def _get_nc():
    if "nc" not in _NC_CACHE:
        _NC_CACHE["nc"] = build()
    return _NC_CACHE["nc"]


def _shard(x: np.ndarray):
    return [
        np.ascontiguousarray(
            x[i * ROWS_PER_CORE: (i + 1) * ROWS_PER_CORE].reshape(NTILES * P, F)
        )
        for i in range(N_CORES)
    ]


def run(x: np.ndarray, trace: bool = False):
    nc = _get_nc()
    in_maps = [{"x": s} for s in _shard(x)]
    res = run_bass_kernel_spmd(nc, in_maps, core_ids=list(range(N_CORES)), trace=trace)
    parts = [
        np.asarray(m["y"]).view(np.uint32).reshape(ROWS_PER_CORE, COLS // 32)
        for m in res.results
    ]
    return np.concatenate(parts, axis=0), res


def kernel(x: np.ndarray) -> np.ndarray:
    out, _ = run(np.asarray(x, dtype=np.float32), trace=False)
    return out


# revision 6
# speedup vs baseline: 1.0264x; 1.0013x over previous
"""Trainium2 bit-packing kernel (ConsolidateBits).

Input : x (4096, 32768) float32, uniform [0,1).
Output: (4096, 1024) uint32 — every 32 consecutive values along the last
dim packed into one word, bit i = (x > 0.5) at offset i.

Sharding: data-parallel over the batch dim, 512 rows per core, 8 cores.

Per-core plan (512 x 32768 f32 = 16 row-tiles of [128 part x 8192]):
  DMA-in : every tile's columns are split across the three DMA queues
           (SP / Act / Pool-SWDGE), each modeled at ~332 GB/s, so arrivals
           advance in lockstep with compute consumption.
  DVE    : 8 scaled compares per compute segment,
           c_j = (x[8s+j] > 0.5) * 2^j -> uint8  (is_gt+mult, 2x_2p).
           c_j[s] is byte s's bit-j contribution. Tiles 1..14 are processed
           as 7 two-tile macro segments to amortize instruction overhead.
  Pool   : radix-2 add tree on int32 views of the uint8 lanes — each int32
           add carries FOUR byte-lanes at once (all partial sums <= 255 so
           no carry ever crosses a lane). The final level's int32 result IS
           the packed little-endian word array.
  DMA-out: one contiguous int32 store per segment, issued with a lag so it
           never blocks a queue head.
"""

import sys

if "/opt/trn_rl_repo" not in sys.path:
    sys.path.insert(0, "/opt/trn_rl_repo")

import numpy as np

import concourse.bass as bass  # noqa: F401
import concourse.bacc as bacc
import concourse.mybir as mybir
from concourse.tile import TileContext
from concourse.alu_op_type import AluOpType as A
from concourse.bass_utils import run_bass_kernel_spmd

P = 128
N_CORES = 8
ROWS, COLS = 4096, 32768
ROWS_PER_CORE = ROWS // N_CORES   # 512
F = 8192                          # free-dim elements per partition per tile
NTILES = (ROWS_PER_CORE * COLS) // (P * F)  # 16

f32 = mybir.dt.float32
i32 = mybir.dt.int32
u8 = mybir.dt.uint8


def default_plan(pool_map, t0_widths, t15_widths, macro):
    """Returns (segments, chunks).

    segments: consumption-ordered (tiles, col0, width, new_xt) where
    `tiles` is the tuple of row-tiles the segment's xt allocation spans
    (compare/tree instructions cover the whole allocation for macro
    segments). col0/width are within the allocation.
    chunks: per segment, list of (alloc_col0, width, queue) DMA pieces.
    """
    segs = []
    chunks = []

    def hw_split(alloc0, tile, tcol, w, pw_q=None):
        """Chunks (alloc_c0, tile, tile_c0, width, queue) covering
        [tcol, tcol+w) of `tile`; optional Pool lead chunk of pw_q cols."""
        ch = []
        if pw_q:
            ch.append((alloc0, tile, tcol, pw_q, "gpsimd"))
            alloc0 += pw_q
            tcol += pw_q
            w -= pw_q
        ch.append((alloc0, tile, tcol, w // 2, "sync"))
        ch.append((alloc0 + w // 2, tile, tcol + w // 2, w - w // 2, "scalar"))
        return ch

    # tile 0: split segments sharing one full-tile allocation
    assert sum(t0_widths) == F
    col = 0
    for w in t0_widths:
        segs.append(((0,), w, col))
        chunks.append(hw_split(col, 0, col, w))
        col += w

    # tiles 1..14: two-tile macro segments (or singles if macro=1)
    t = 1
    while t < NTILES - 1:
        span = min(macro, NTILES - 1 - t)
        tiles = tuple(range(t, t + span))
        segs.append((tiles, span * F, 0))
        ch = []
        for k, tt in enumerate(tiles):
            pw = pool_map.get(tt, 0)
            ch += hw_split(k * F, tt, 0, F, pw or None)
        chunks.append(ch)
        t += span

    # tile 15: split segments sharing one full-tile allocation
    assert sum(t15_widths) == F
    col = 0
    for w in t15_widths:
        segs.append(((15,), w, col))
        chunks.append(hw_split(col, 15, col, w))
        col += w
    return segs, chunks


def build(
    dve_trees=(),            # segment indices with tree on DVE
    pool_cmps=(20, 21),      # segment indices with compares on Pool
    out_lag=3,               # out-DMA issued this many segments late
    lead=4,                  # in-DMA issued this many segments early
    pool_tiles=(3, 4, 5, 6, 7, 8, 9, 10, 11, 12, 13, 14),
    pool_width=2176,
    pool_head={1: 4096, 2: 4096},
    tail_outs_on_pool=0,
    t0_widths=(2048, 2048, 2048, 2048),
    t15_widths=(4096, 2304, 1664, 128),
    macro=1,
    out_batch=2,
    xt_bufs=4,
    c_bufs=3,
    mid_bufs=2,
    b_bufs=6,
    pool_map=None,
):
    if pool_map is None:
        pool_map = dict(pool_head)
        pool_map.update({t: pool_width for t in pool_tiles})
    else:
        pool_map = {int(k): v for k, v in pool_map.items()}
    segs, chunks = default_plan(pool_map, tuple(t0_widths),
                                tuple(t15_widths), macro)
    n_seg = len(segs)

    nc = bacc.Bacc("TRN2", target_bir_lowering=False)
    x = nc.dram_tensor("x", [NTILES * P, F], f32, kind="ExternalInput")
    y = nc.dram_tensor("y", [NTILES * P, F // 32], i32, kind="ExternalOutput")
    xr = x[:, :].rearrange("(t p) f -> t p f", p=P)
    yr = y[:, :].rearrange("(t p) w -> t p w", p=P)

    with TileContext(nc) as tc:
        with (
            tc.tile_pool(name="xt", bufs=xt_bufs) as xt_pool,
            tc.tile_pool(name="c", bufs=c_bufs) as c_pool,
            tc.tile_pool(name="mid", bufs=mid_bufs) as mid_pool,
            tc.tile_pool(name="b", bufs=b_bufs) as b_pool,
        ):
            xts = [None] * n_seg
            shared_xt = {}
            outs = [None] * n_seg
            pending_b = [None]
            out_rr = 0

            def issue_in(si):
                tiles, fw, tcol = segs[si]
                if tiles[0] not in (0, NTILES - 1):
                    xt = xt_pool.tile([P, len(tiles) * F], f32, tag="xt",
                                      name=f"xt_s{si}")
                else:
                    # head/tail tiles: one shared full-tile allocation;
                    # segments DMA into their column ranges
                    t = tiles[0]
                    if t not in shared_xt:
                        shared_xt[t] = xt_pool.tile(
                            [P, F], f32, tag="xt", name=f"xth{t}")
                    xt = shared_xt[t]
                for c0, tt, tc0, w, q in chunks[si]:
                    getattr(nc, q).dma_start(
                        xt[:, c0:c0 + w], xr[tt][:, tc0:tc0 + w])
                xts[si] = xt

            def issue_out(si):
                if outs[si] is None:
                    return
                b, tiles, tcol, nw, q = outs[si]
                if len(tiles) == 1:
                    dst = yr[tiles[0]][:, tcol // 32: tcol // 32 + nw]
                    src = b[:]
                else:
                    h = len(tiles)
                    dst = y[P * tiles[0]: P * (tiles[0] + h), :].rearrange(
                        "(h p) w -> p h w", p=P)
                    src = b[:].rearrange("p (h w) -> p h w", h=h)
                getattr(nc, q).dma_start(dst, src)

            for si in range(min(lead, n_seg)):
                issue_in(si)

            for si in range(n_seg):
                tiles, fw, tcol = segs[si]
                nb = fw // 8
                nw = fw // 32
                xt = xts[si]
                if tiles[0] in (0, NTILES - 1):
                    xv = xt[:, tcol:tcol + fw].rearrange(
                        "p (s m) -> p s m", m=8)
                else:
                    xv = xt[:].rearrange("p (s m) -> p s m", m=8)

                # 8 scaled compares: c_j = (x[8s+j] > 0.5) * 2^j  -> uint8
                c = c_pool.tile([P, 8 * nb], u8, tag="c", name=f"c{si}")
                cv = c[:].rearrange("p (j s) -> p j s", j=8)
                cmp_eng = nc.gpsimd if si in pool_cmps else nc.vector
                for j in range(8):
                    cmp_eng.tensor_scalar(
                        out=cv[:, j], in0=xv[:, :, j],
                        scalar1=0.5, scalar2=float(1 << j),
                        op0=A.is_gt, op1=A.mult,
                    )

                # add tree on int32 views (4 uint8 byte-lanes per element);
                # the L3 int32 result IS the packed little-endian word array
                eng = nc.vector if si in dve_trees else nc.gpsimd
                ci = c[:].bitcast(i32).rearrange("p (j s) -> p j s", j=8)
                t1 = mid_pool.tile([P, nb], i32, tag="t1", name=f"t1_{si}")
                t1v = t1[:].rearrange("p (j s) -> p j s", j=4)
                eng.tensor_tensor(
                    out=t1v, in0=ci[:, 0:8:2], in1=ci[:, 1:8:2], op=A.add)
                t2 = mid_pool.tile([P, nb // 2], i32, tag="t2", name=f"t2_{si}")
                t2v = t2[:].rearrange("p (j s) -> p j s", j=2)
                t1b = t1[:].rearrange("p (j s) -> p j s", j=4)
                eng.tensor_tensor(
                    out=t2v, in0=t1b[:, 0:4:2], in1=t1b[:, 1:4:2], op=A.add)
                # mid tiles: pair consecutive tiles into one shared b tile
                # so a single out-DMA stores both (y rows are contiguous)
                mid = tiles[0] not in (0, NTILES - 1)
                pair = (out_batch == 2 and mid and len(tiles) == 1
                        and 1 <= tiles[0] <= 14)
                if pair and tiles[0] % 2 == 1:
                    b = b_pool.tile([P, 2 * nw], i32, tag="b", name=f"b{si}")
                    pending_b[0] = b
                    bslice = b[:, 0:nw]
                elif pair:
                    b = pending_b[0]
                    bslice = b[:, nw:2 * nw]
                else:
                    b = b_pool.tile([P, nw], i32, tag="b", name=f"b{si}")
                    bslice = b[:]
                t2b = t2[:].rearrange("p (j s) -> p j s", j=2)
                eng.tensor_tensor(
                    out=bslice.rearrange("p (one s) -> p one s", one=1),
                    in0=t2b[:, 0:1], in1=t2b[:, 1:2], op=A.add)

                if si >= n_seg - tail_outs_on_pool:
                    oq = "gpsimd"
                else:
                    oq = "sync" if out_rr % 2 == 0 else "scalar"
                if pair and tiles[0] % 2 == 1:
                    outs[si] = None          # store happens with the pair's 2nd
                else:
                    out_rr += 1
                    if pair:
                        outs[si] = (b, (tiles[0] - 1, tiles[0]), 0, 2 * nw, oq)
                    else:
                        outs[si] = (b, tiles, tcol, nw, oq)

                if si + lead < n_seg:
                    issue_in(si + lead)
                if si >= out_lag:
                    issue_out(si - out_lag)

            for si in range(n_seg - out_lag, n_seg):
                issue_out(si)

    nc.compile()
    return nc


_NC_CACHE = {}


def _get_nc():
    if "nc" not in _NC_CACHE:
        _NC_CACHE["nc"] = build()
    return _NC_CACHE["nc"]


def _shard(x: np.ndarray):
    return [
        np.ascontiguousarray(
            x[i * ROWS_PER_CORE: (i + 1) * ROWS_PER_CORE].reshape(NTILES * P, F)
        )
        for i in range(N_CORES)
    ]


def run(x: np.ndarray, trace: bool = False):
    nc = _get_nc()
    in_maps = [{"x": s} for s in _shard(x)]
    res = run_bass_kernel_spmd(nc, in_maps, core_ids=list(range(N_CORES)), trace=trace)
    parts = [
        np.asarray(m["y"]).view(np.uint32).reshape(ROWS_PER_CORE, COLS // 32)
        for m in res.results
    ]
    return np.concatenate(parts, axis=0), res


def kernel(x: np.ndarray) -> np.ndarray:
    out, _ = run(np.asarray(x, dtype=np.float32), trace=False)
    return out


# revision 7
# speedup vs baseline: 1.0265x; 1.0001x over previous
"""Trainium2 bit-packing kernel (ConsolidateBits).

Input : x (4096, 32768) float32, uniform [0,1).
Output: (4096, 1024) uint32 — every 32 consecutive values along the last
dim packed into one word, bit i = (x > 0.5) at offset i.

Sharding: data-parallel over the batch dim, 512 rows per core, 8 cores.

Per-core plan (512 x 32768 f32 = 16 row-tiles of [128 part x 8192]):
  DMA-in : every tile's columns are split across the three DMA queues
           (SP / Act / Pool-SWDGE), each modeled at ~332 GB/s, so arrivals
           advance in lockstep with compute consumption.
  DVE    : 8 scaled compares per compute segment,
           c_j = (x[8s+j] > 0.5) * 2^j -> uint8  (is_gt+mult, 2x_2p).
           c_j[s] is byte s's bit-j contribution. Tiles 1..14 are processed
           as 7 two-tile macro segments to amortize instruction overhead.
  Pool   : radix-2 add tree on int32 views of the uint8 lanes — each int32
           add carries FOUR byte-lanes at once (all partial sums <= 255 so
           no carry ever crosses a lane). The final level's int32 result IS
           the packed little-endian word array.
  DMA-out: one contiguous int32 store per segment, issued with a lag so it
           never blocks a queue head.
"""

import sys

if "/opt/trn_rl_repo" not in sys.path:
    sys.path.insert(0, "/opt/trn_rl_repo")

import numpy as np

import concourse.bass as bass  # noqa: F401
import concourse.bacc as bacc
import concourse.mybir as mybir
from concourse.tile import TileContext
from concourse.alu_op_type import AluOpType as A
from concourse.bass_utils import run_bass_kernel_spmd

P = 128
N_CORES = 8
ROWS, COLS = 4096, 32768
ROWS_PER_CORE = ROWS // N_CORES   # 512
F = 8192                          # free-dim elements per partition per tile
NTILES = (ROWS_PER_CORE * COLS) // (P * F)  # 16

f32 = mybir.dt.float32
i32 = mybir.dt.int32
u8 = mybir.dt.uint8


def default_plan(pool_map, t0_widths, t15_widths, macro):
    """Returns (segments, chunks).

    segments: consumption-ordered (tiles, col0, width, new_xt) where
    `tiles` is the tuple of row-tiles the segment's xt allocation spans
    (compare/tree instructions cover the whole allocation for macro
    segments). col0/width are within the allocation.
    chunks: per segment, list of (alloc_col0, width, queue) DMA pieces.
    """
    segs = []
    chunks = []

    def hw_split(alloc0, tile, tcol, w, pw_q=None):
        """Chunks (alloc_c0, tile, tile_c0, width, queue) covering
        [tcol, tcol+w) of `tile`; optional Pool lead chunk of pw_q cols."""
        ch = []
        if pw_q:
            ch.append((alloc0, tile, tcol, pw_q, "gpsimd"))
            alloc0 += pw_q
            tcol += pw_q
            w -= pw_q
        ch.append((alloc0, tile, tcol, w // 2, "sync"))
        ch.append((alloc0 + w // 2, tile, tcol + w // 2, w - w // 2, "scalar"))
        return ch

    # tile 0: split segments sharing one full-tile allocation
    assert sum(t0_widths) == F
    col = 0
    for w in t0_widths:
        segs.append(((0,), w, col))
        chunks.append(hw_split(col, 0, col, w))
        col += w

    # tiles 1..14: two-tile macro segments (or singles if macro=1)
    t = 1
    while t < NTILES - 1:
        span = min(macro, NTILES - 1 - t)
        tiles = tuple(range(t, t + span))
        segs.append((tiles, span * F, 0))
        ch = []
        for k, tt in enumerate(tiles):
            pw = pool_map.get(tt, 0)
            ch += hw_split(k * F, tt, 0, F, pw or None)
        chunks.append(ch)
        t += span

    # tile 15: split segments sharing one full-tile allocation
    assert sum(t15_widths) == F
    col = 0
    for w in t15_widths:
        segs.append(((15,), w, col))
        chunks.append(hw_split(col, 15, col, w))
        col += w
    return segs, chunks


def build(
    dve_trees=(),            # segment indices with tree on DVE
    pool_cmps=(20, 21),      # segment indices with compares on Pool
    out_lag=3,               # out-DMA issued this many segments late
    lead=4,                  # in-DMA issued this many segments early
    pool_tiles=(3, 4, 5, 6, 7, 8, 9, 10, 11, 12, 13, 14),
    pool_width=2176,
    pool_head={1: 4096, 2: 4096},
    tail_outs_on_pool=0,
    t0_widths=(2048, 2048, 2176, 1920),
    t15_widths=(4096, 2304, 1664, 128),
    macro=1,
    out_batch=2,
    xt_bufs=4,
    c_bufs=3,
    mid_bufs=2,
    b_bufs=6,
    pool_map=None,
):
    if pool_map is None:
        pool_map = dict(pool_head)
        pool_map.update({t: pool_width for t in pool_tiles})
    else:
        pool_map = {int(k): v for k, v in pool_map.items()}
    segs, chunks = default_plan(pool_map, tuple(t0_widths),
                                tuple(t15_widths), macro)
    n_seg = len(segs)

    nc = bacc.Bacc("TRN2", target_bir_lowering=False)
    x = nc.dram_tensor("x", [NTILES * P, F], f32, kind="ExternalInput")
    y = nc.dram_tensor("y", [NTILES * P, F // 32], i32, kind="ExternalOutput")
    xr = x[:, :].rearrange("(t p) f -> t p f", p=P)
    yr = y[:, :].rearrange("(t p) w -> t p w", p=P)

    with TileContext(nc) as tc:
        with (
            tc.tile_pool(name="xt", bufs=xt_bufs) as xt_pool,
            tc.tile_pool(name="c", bufs=c_bufs) as c_pool,
            tc.tile_pool(name="mid", bufs=mid_bufs) as mid_pool,
            tc.tile_pool(name="b", bufs=b_bufs) as b_pool,
        ):
            xts = [None] * n_seg
            shared_xt = {}
            outs = [None] * n_seg
            pending_b = [None]
            out_rr = 0

            def issue_in(si):
                tiles, fw, tcol = segs[si]
                if tiles[0] not in (0, NTILES - 1):
                    xt = xt_pool.tile([P, len(tiles) * F], f32, tag="xt",
                                      name=f"xt_s{si}")
                else:
                    # head/tail tiles: one shared full-tile allocation;
                    # segments DMA into their column ranges
                    t = tiles[0]
                    if t not in shared_xt:
                        shared_xt[t] = xt_pool.tile(
                            [P, F], f32, tag="xt", name=f"xth{t}")
                    xt = shared_xt[t]
                for c0, tt, tc0, w, q in chunks[si]:
                    getattr(nc, q).dma_start(
                        xt[:, c0:c0 + w], xr[tt][:, tc0:tc0 + w])
                xts[si] = xt

            def issue_out(si):
                if outs[si] is None:
                    return
                b, tiles, tcol, nw, q = outs[si]
                if len(tiles) == 1:
                    dst = yr[tiles[0]][:, tcol // 32: tcol // 32 + nw]
                    src = b[:]
                else:
                    h = len(tiles)
                    dst = y[P * tiles[0]: P * (tiles[0] + h), :].rearrange(
                        "(h p) w -> p h w", p=P)
                    src = b[:].rearrange("p (h w) -> p h w", h=h)
                getattr(nc, q).dma_start(dst, src)

            for si in range(min(lead, n_seg)):
                issue_in(si)

            for si in range(n_seg):
                tiles, fw, tcol = segs[si]
                nb = fw // 8
                nw = fw // 32
                xt = xts[si]
                if tiles[0] in (0, NTILES - 1):
                    xv = xt[:, tcol:tcol + fw].rearrange(
                        "p (s m) -> p s m", m=8)
                else:
                    xv = xt[:].rearrange("p (s m) -> p s m", m=8)

                # 8 scaled compares: c_j = (x[8s+j] > 0.5) * 2^j  -> uint8
                c = c_pool.tile([P, 8 * nb], u8, tag="c", name=f"c{si}")
                cv = c[:].rearrange("p (j s) -> p j s", j=8)
                cmp_eng = nc.gpsimd if si in pool_cmps else nc.vector
                for j in range(8):
                    cmp_eng.tensor_scalar(
                        out=cv[:, j], in0=xv[:, :, j],
                        scalar1=0.5, scalar2=float(1 << j),
                        op0=A.is_gt, op1=A.mult,
                    )

                # add tree on int32 views (4 uint8 byte-lanes per element);
                # the L3 int32 result IS the packed little-endian word array
                eng = nc.vector if si in dve_trees else nc.gpsimd
                ci = c[:].bitcast(i32).rearrange("p (j s) -> p j s", j=8)
                t1 = mid_pool.tile([P, nb], i32, tag="t1", name=f"t1_{si}")
                t1v = t1[:].rearrange("p (j s) -> p j s", j=4)
                eng.tensor_tensor(
                    out=t1v, in0=ci[:, 0:8:2], in1=ci[:, 1:8:2], op=A.add)
                t2 = mid_pool.tile([P, nb // 2], i32, tag="t2", name=f"t2_{si}")
                t2v = t2[:].rearrange("p (j s) -> p j s", j=2)
                t1b = t1[:].rearrange("p (j s) -> p j s", j=4)
                eng.tensor_tensor(
                    out=t2v, in0=t1b[:, 0:4:2], in1=t1b[:, 1:4:2], op=A.add)
                # mid tiles: pair consecutive tiles into one shared b tile
                # so a single out-DMA stores both (y rows are contiguous)
                mid = tiles[0] not in (0, NTILES - 1)
                pair = (out_batch == 2 and mid and len(tiles) == 1
                        and 1 <= tiles[0] <= 14)
                if pair and tiles[0] % 2 == 1:
                    b = b_pool.tile([P, 2 * nw], i32, tag="b", name=f"b{si}")
                    pending_b[0] = b
                    bslice = b[:, 0:nw]
                elif pair:
                    b = pending_b[0]
                    bslice = b[:, nw:2 * nw]
                else:
                    b = b_pool.tile([P, nw], i32, tag="b", name=f"b{si}")
                    bslice = b[:]
                t2b = t2[:].rearrange("p (j s) -> p j s", j=2)
                eng.tensor_tensor(
                    out=bslice.rearrange("p (one s) -> p one s", one=1),
                    in0=t2b[:, 0:1], in1=t2b[:, 1:2], op=A.add)

                if si >= n_seg - tail_outs_on_pool:
                    oq = "gpsimd"
                else:
                    oq = "sync" if out_rr % 2 == 0 else "scalar"
                if pair and tiles[0] % 2 == 1:
                    outs[si] = None          # store happens with the pair's 2nd
                else:
                    out_rr += 1
                    if pair:
                        outs[si] = (b, (tiles[0] - 1, tiles[0]), 0, 2 * nw, oq)
                    else:
                        outs[si] = (b, tiles, tcol, nw, oq)

                if si + lead < n_seg:
                    issue_in(si + lead)
                if si >= out_lag:
                    issue_out(si - out_lag)

            for si in range(n_seg - out_lag, n_seg):
                issue_out(si)

    nc.compile()
    return nc


_NC_CACHE = {}


def _get_nc():
    if "nc" not in _NC_CACHE:
        _NC_CACHE["nc"] = build()
    return _NC_CACHE["nc"]


def _shard(x: np.ndarray):
    return [
        np.ascontiguousarray(
            x[i * ROWS_PER_CORE: (i + 1) * ROWS_PER_CORE].reshape(NTILES * P, F)
        )
        for i in range(N_CORES)
    ]


def run(x: np.ndarray, trace: bool = False):
    nc = _get_nc()
    in_maps = [{"x": s} for s in _shard(x)]
    res = run_bass_kernel_spmd(nc, in_maps, core_ids=list(range(N_CORES)), trace=trace)
    parts = [
        np.asarray(m["y"]).view(np.uint32).reshape(ROWS_PER_CORE, COLS // 32)
        for m in res.results
    ]
    return np.concatenate(parts, axis=0), res


def kernel(x: np.ndarray) -> np.ndarray:
    out, _ = run(np.asarray(x, dtype=np.float32), trace=False)
    return out
